# revision 2
# baseline (speedup 1.0000x reference)
"""Trainium2 Bass kernel for nn_Critique: alternating max/min scan of 128
planes d_k[c,i,j] = (i-p0)n0 + (j-p1)n1 + (c-p2)n2 over a [3,1024,1024] grid.

Sharding: W split across 8 cores (128 columns each). Per-core layout:
partition = j (128), free = (c,i) = 3*1024. Per plane step, a generator
engine materialises D = a_k * i + bias[j,c,k] (ScalarE activation with
per-partition scale/bias APs, or VectorE tensor_scalar at 2x rate), then
VectorE applies carry = max/min(carry, D).

A host-side exact analysis prunes each plane to a covering (c, i-interval):
the scan is simulated on a sampled grid in float64, and the carry/relevance
bounds between samples are guarded by the planes' Lipschitz constants plus
a margin. Applying a plane outside its true support is a no-op, so covering
intervals preserve exactness. One SPMD program is shared by all 8 cores, so
the schedule is the union over cores.
"""
import numpy as np
from contextlib import ExitStack

import concourse.bass as bass
import concourse.tile as tile
from concourse import mybir, bacc
from concourse.bass_utils import run_bass_kernel_spmd

H, W, C, N = 1024, 1024, 3, 128
NCORES = 8
SHARD = W // NCORES          # 128 j-columns per core
CH = 32                      # pruning chunk width along i
NCH = H // CH
SI = 8                       # host sample stride along i
SJ = 8                       # host sample stride along j
MARGIN = 0.1                 # guards fp32-vs-f64 plane evaluation error
FREE = C * H                 # 3072

# measured per-op cost model (ns) for the generator assignment
_ACT_GEN = lambda w: 282 + 0.833 * w
_DVE_TT = lambda w: 110 + 1.042 * w
_DVE_TS = lambda w: 110 + 0.521 * w


def _sample_pruning(bp, nv, j0, j1):
    """survive[k, c, chunk]: forward reachability + backward relevance from
    an exact float64 scan on a sampled grid, Lipschitz-guarded."""
    n0, n1, n2 = nv[:, 0], nv[:, 1], nv[:, 2]
    p0, p1, p2 = bp[:, 0], bp[:, 1], bp[:, 2]
    cs = np.arange(C)
    slack = (np.abs(n0).max() * (SI / 2) + np.abs(n1).max() * (SJ / 2)
             + MARGIN)
    iis = np.arange(SI // 2, H, SI)
    jjs = np.arange(j0 + SJ // 2, j1, SJ)
    spc = len(iis) // NCH
    i_lo = np.arange(NCH) * CH
    i_hi = i_lo + CH - 1
    ti = np.stack([(i_lo[None, :] - p0[:, None]) * n0[:, None],
                   (i_hi[None, :] - p0[:, None]) * n0[:, None]])
    ti_lo, ti_hi = ti.min(0), ti.max(0)
    tj = np.stack([(j0 - p1) * n1, (j1 - 1 - p1) * n1])
    tj_lo, tj_hi = tj.min(0), tj.max(0)
    tc = (cs[None, :] - p2[:, None]) * n2[:, None]
    dlo_ch = ti_lo[:, None, :] + tj_lo[:, None, None] + tc[:, :, None]
    dhi_ch = ti_hi[:, None, :] + tj_hi[:, None, None] + tc[:, :, None]

    def dk(k):
        return (((iis - p0[k]) * n0[k])[None, :, None]
                + ((jjs - p1[k]) * n1[k])[None, None, :]
                + ((cs - p2[k]) * n2[k])[:, None, None])

    carry = np.full((C, len(iis), len(jjs)), -np.inf)
    fwd = np.zeros((N, C, NCH), bool)
    for k in range(N):
        d = dk(k)
        if k % 2 == 0:
            cmin = carry.reshape(C, NCH, spc, -1).min(axis=(2, 3)) - slack
            fwd[k] = ~(dhi_ch[k] <= cmin)
            carry = np.maximum(carry, d)
        else:
            cmax = carry.reshape(C, NCH, spc, -1).max(axis=(2, 3)) + slack
            fwd[k] = ~(dlo_ch[k] >= cmax)
            carry = np.minimum(carry, d)
    A = np.full_like(carry, -np.inf)
    B = np.full_like(carry, np.inf)
    bwd = np.zeros((N, C, NCH), bool)
    for k in range(N - 1, -1, -1):
        d = dk(k)
        if k % 2 == 0:
            Amin = A.reshape(C, NCH, spc, -1).min(axis=(2, 3)) - slack
            bwd[k] = ~(dhi_ch[k] <= Amin)
            A = np.minimum(np.maximum(d, A), B)
        else:
            Bmax = B.reshape(C, NCH, spc, -1).max(axis=(2, 3)) + slack
            bwd[k] = ~(dlo_ch[k] >= Bmax)
            B = np.minimum(np.maximum(d, A), B)
    return fwd & bwd


def _schedule(bp64, nv64):
    """Union-over-cores op list.

    Returns a list of per-plane entries (k, gi0, gw, [(c, i0, w), ...]):
    gi0/gw is the i-extent of the shared generated plane E_k (union of the
    per-c covers), and each (c, i0, w) is one fused scalar_tensor_tensor
    apply of E_k + shift_c to carry's c-block.
    """
    surv = np.zeros((N, C, NCH), bool)
    for core in range(NCORES):
        surv |= _sample_pruning(bp64, nv64, core * SHARD, (core + 1) * SHARD)
    surv[0] = True  # k=0 writes carry directly; force full cover
    plan = []
    act_t = dve_t = 0.0
    for k in range(N):
        cs = []
        for c in range(C):
            idx = np.nonzero(surv[k, c])[0]
            if len(idx) == 0:
                continue
            i0 = idx[0] * CH
            w = (idx[-1] + 1) * CH - i0
            cs.append((c, i0, w))
        if not cs:
            continue
        gi0 = min(i0 for _, i0, _ in cs)
        gend = max(i0 + w for _, i0, w in cs)
        gw = gend - gi0
        if k == 0:
            act_t += sum(282 + 0.833 * w for _, _, w in cs)
            plan.append((k, "gen3", gi0, gw, cs))
            continue
        # mode gen3: per-c ACT gen + DVE tensor_tensor
        a_act = sum(300 + 0.87 * w for _, _, w in cs)
        a_dve = sum(100 + 1.042 * w for _, _, w in cs)
        # mode gen1: one shared ACT gen of E_k + per-c DVE scalar_tensor_tensor
        b_act = 300 + 0.87 * gw
        b_dve = sum(238 + 1.042 * w for _, _, w in cs)
        if max(act_t + a_act, dve_t + a_dve) <= max(act_t + b_act, dve_t + b_dve):
            act_t += a_act
            dve_t += a_dve
            plan.append((k, "gen3", gi0, gw, cs))
        else:
            act_t += b_act
            dve_t += b_dve
            plan.append((k, "gen1", gi0, gw, cs))
    return plan


def _build(plan, cshift):
    """cshift[k, c] = fp32 (c - p2_k) * n2_k, baked as immediates."""
    nc = bacc.Bacc("TRN2", target_bir_lowering=False, debug=False)
    acol_d = nc.dram_tensor("acol", [128, N], mybir.dt.float32, kind="ExternalInput")
    pp_d = nc.dram_tensor("pp", [128, N], mybir.dt.float32, kind="ExternalInput")
    ppc_d = nc.dram_tensor("ppc", [128, C * N], mybir.dt.float32, kind="ExternalInput")
    out_d = nc.dram_tensor("out", [128, FREE], mybir.dt.float32, kind="ExternalOutput")

    with ExitStack() as ctx:
        tc = ctx.enter_context(tile.TileContext(nc))
        pool = ctx.enter_context(tc.tile_pool(name="main", bufs=1))
        dpool = ctx.enter_context(tc.tile_pool(name="dgen", bufs=12))

        carry = pool.tile([128, FREE], mybir.dt.float32)
        acol = pool.tile([128, N], mybir.dt.float32)
        pp = pool.tile([128, N], mybir.dt.float32)
        ppc = pool.tile([128, C * N], mybir.dt.float32)
        ioti = pool.tile([128, H], mybir.dt.int32)
        iotf = pool.tile([128, H], mybir.dt.float32)
        nc.gpsimd.dma_start(acol[:], acol_d[:])
        nc.gpsimd.dma_start(pp[:], pp_d[:])
        nc.gpsimd.dma_start(ppc[:], ppc_d[:])
        nc.gpsimd.iota(ioti[:], pattern=[[1, H]], base=0, channel_multiplier=0)
        nc.scalar.copy(iotf[:], ioti[:])

        ident = mybir.ActivationFunctionType.Identity
        for (k, mode, gi0, gw, cs) in plan:
            sc = acol[:, k:k + 1]
            op = mybir.AluOpType.max if k % 2 == 0 else mybir.AluOpType.min
            if k == 0:
                # max(-inf, D) = D: generate straight into carry per c
                for (c, i0, w) in cs:
                    nc.scalar.activation(carry[:, c * H + i0: c * H + i0 + w],
                                         iotf[:, i0:i0 + w], ident,
                                         bias=ppc[:, c * N: c * N + 1], scale=sc)
                continue
            if mode == "gen3":
                for (c, i0, w) in cs:
                    f0 = c * H + i0
                    dbuf = dpool.tile([128, H], mybir.dt.float32, tag="ebuf")
                    nc.scalar.activation(dbuf[:, :w], iotf[:, i0:i0 + w],
                                         ident, bias=ppc[:, c * N + k: c * N + k + 1],
                                         scale=sc)
                    nc.vector.tensor_tensor(carry[:, f0:f0 + w],
                                            carry[:, f0:f0 + w], dbuf[:, :w], op)
            else:
                ebuf = dpool.tile([128, H], mybir.dt.float32, tag="ebuf")
                nc.scalar.activation(ebuf[:, :gw], iotf[:, gi0:gi0 + gw], ident,
                                     bias=pp[:, k:k + 1], scale=sc)
                for (c, i0, w) in cs:
                    f0 = c * H + i0
                    e0 = i0 - gi0
                    nc.vector.scalar_tensor_tensor(
                        carry[:, f0:f0 + w], ebuf[:, e0:e0 + w],
                        float(cshift[k, c]), carry[:, f0:f0 + w],
                        mybir.AluOpType.add, op)

        nc.gpsimd.dma_start(out_d[:], carry[:])
    nc.compile()
    return nc


def _prepare(basepoints, normal_vectors):
    bp = np.asarray(basepoints, np.float32)
    nv = np.asarray(normal_vectors, np.float32)
    bp64 = bp.astype(np.float64)
    nv64 = nv.astype(np.float64)

    plan = _schedule(bp64, nv64)
    # cshift[k, c] = (c - p2_k) * n2_k in float64, rounded to fp32
    cshift = ((np.arange(C)[None, :] - bp64[:, 2:3]) * nv64[:, 2:3]).astype(np.float32)
    nc = _build(plan, cshift)

    acol = np.broadcast_to(nv[:, 0][None, :], (128, N)).copy()  # a_k = n0_k
    js = np.arange(W, dtype=np.float32)
    in_maps = []
    for core in range(NCORES):
        j = js[core * SHARD:(core + 1) * SHARD]                   # [128]
        B = (j[:, None] - bp[None, :, 1]) * nv[None, :, 1]        # [128, N]
        # device computes E = a_k * i + q, D_c = E + cshift[k,c]; fold the
        # -p0*n0 of the reference's (i - p0)*n0 into q in float64 to
        # minimise the association difference vs the reference.
        corr = (-bp64[:, 0] * nv64[:, 0])[None, :]
        q = (B.astype(np.float64) + corr).astype(np.float32)      # [128, N]
        ppc = q[:, None, :] + cshift.T[None, :, :]                # [128, C, N]
        in_maps.append({
            "acol": acol,
            "pp": np.ascontiguousarray(q),
            "ppc": np.ascontiguousarray(ppc.reshape(128, C * N)),
        })
    return nc, in_maps


def _gather(res):
    out = np.empty((C, H, W), np.float32)
    for core in range(NCORES):
        o = res.results[core]["out"].reshape(SHARD, C, H)  # [j, c, i]
        out[:, :, core * SHARD:(core + 1) * SHARD] = o.transpose(1, 2, 0)
    return out


def kernel(basepoints: np.ndarray, normal_vectors: np.ndarray) -> np.ndarray:
    nc, in_maps = _prepare(basepoints, normal_vectors)
    res = run_bass_kernel_spmd(nc, in_maps, list(range(NCORES)))
    return _gather(res)


def kernel_timed(basepoints: np.ndarray, normal_vectors: np.ndarray):
    """Run with NTFF tracing; returns (exec_time_ns, output)."""
    nc, in_maps = _prepare(basepoints, normal_vectors)
    res = run_bass_kernel_spmd(nc, in_maps, list(range(NCORES)), trace=True,
                               trace_cores=list(range(NCORES)))
    return res.exec_time_ns, _gather(res), res



# revision 5
# speedup vs baseline: 1.1122x; 1.1122x over previous
"""Trainium2 Bass kernel for nn_Critique: alternating max/min scan of 128
planes d_k[c,i,j] = (i-p0)n0 + (j-p1)n1 + (c-p2)n2 over a [3,1024,1024] grid.

Sharding: W split across 8 cores (128 j-columns each). Per-core layout:
partition = j (128), carry [128, 3, 1024] in fp16 (free = (c, i)).

The max/min scan is a strict dependency chain through carry, so applies
cannot overlap each other on one engine. The grid is split at a fixed
i-boundary ISTAR into two independent chains: DVE applies i < ISTAR,
GpSimd applies i >= ISTAR (per-voxel ops commute across disjoint regions).
Applies are fused 3-channel tensor_tensor ops in fp16 (DVE 2x mode).

Plane generation (D_kc[j,i] = a_k*i + ppc[j,c,k], fp16 out) is split
between ScalarE activation (fp32 iota input, per-partition scale/bias) and
DVE tensor_scalar (fp16 iota * imm + per-partition bias AP, 4x mode) to
balance engine loads against the apply chains.

A host-side exact analysis prunes each plane to a covering i-interval
(union over channels and cores; applying a plane outside its true support
is a no-op, and fp16 rounding keeps mispruning error at the noise floor).
Output returns fp16 and is upcast to fp32 on the host (exact).
"""
import numpy as np
from contextlib import ExitStack

import concourse.bass as bass
import concourse.tile as tile
from concourse import mybir, bacc
from concourse.bass_utils import run_bass_kernel_spmd

H, W, C, N = 1024, 1024, 3, 128
NCORES = 8
SHARD = W // NCORES          # 128 j-columns per core
CH = 32                      # pruning chunk width along i
NCH = H // CH
SI = 8                       # host sample stride along i
SJ = 8                       # host sample stride along j
MARGIN = 0.5                 # guards fp16-vs-f64 plane evaluation error

# TimelineSim-fitted per-op costs (ns); used only for load balancing
ACT_BASE, ACT_R = 304.0, 1.00      # activation gen, per (k,c)
TS_BASE, TS_R = 260.0, 0.26        # DVE tensor_scalar gen fp16, per (k,c)
TTV_BASE, TTV_R = 214.0, 0.593     # DVE fused3 tensor_tensor fp16, per plane
TTG_BASE, TTG_R = 550.0, 1.60      # GpSimd fused3 tensor_tensor fp16, per plane


def _sample_pruning(bp, nv, j0, j1):
    """survive[k, c, chunk]: forward reachability + backward relevance from
    an exact float64 scan on a sampled grid, Lipschitz-guarded."""
    n0, n1, n2 = nv[:, 0], nv[:, 1], nv[:, 2]
    p0, p1, p2 = bp[:, 0], bp[:, 1], bp[:, 2]
    cs = np.arange(C)
    slack = (np.abs(n0).max() * (SI / 2) + np.abs(n1).max() * (SJ / 2)
             + MARGIN)
    iis = np.arange(SI // 2, H, SI)
    jjs = np.arange(j0 + SJ // 2, j1, SJ)
    spc = len(iis) // NCH
    i_lo = np.arange(NCH) * CH
    i_hi = i_lo + CH - 1
    ti = np.stack([(i_lo[None, :] - p0[:, None]) * n0[:, None],
                   (i_hi[None, :] - p0[:, None]) * n0[:, None]])
    ti_lo, ti_hi = ti.min(0), ti.max(0)
    tj = np.stack([(j0 - p1) * n1, (j1 - 1 - p1) * n1])
    tj_lo, tj_hi = tj.min(0), tj.max(0)
    tc = (cs[None, :] - p2[:, None]) * n2[:, None]
    dlo_ch = ti_lo[:, None, :] + tj_lo[:, None, None] + tc[:, :, None]
    dhi_ch = ti_hi[:, None, :] + tj_hi[:, None, None] + tc[:, :, None]

    def dk(k):
        return (((iis - p0[k]) * n0[k])[None, :, None]
                + ((jjs - p1[k]) * n1[k])[None, None, :]
                + ((cs - p2[k]) * n2[k])[:, None, None])

    carry = np.full((C, len(iis), len(jjs)), -np.inf)
    fwd = np.zeros((N, C, NCH), bool)
    for k in range(N):
        d = dk(k)
        if k % 2 == 0:
            cmin = carry.reshape(C, NCH, spc, -1).min(axis=(2, 3)) - slack
            fwd[k] = ~(dhi_ch[k] <= cmin)
            carry = np.maximum(carry, d)
        else:
            cmax = carry.reshape(C, NCH, spc, -1).max(axis=(2, 3)) + slack
            fwd[k] = ~(dlo_ch[k] >= cmax)
            carry = np.minimum(carry, d)
    A = np.full_like(carry, -np.inf)
    B = np.full_like(carry, np.inf)
    bwd = np.zeros((N, C, NCH), bool)
    for k in range(N - 1, -1, -1):
        d = dk(k)
        if k % 2 == 0:
            Amin = A.reshape(C, NCH, spc, -1).min(axis=(2, 3)) - slack
            bwd[k] = ~(dhi_ch[k] <= Amin)
            A = np.minimum(np.maximum(d, A), B)
        else:
            Bmax = B.reshape(C, NCH, spc, -1).max(axis=(2, 3)) + slack
            bwd[k] = ~(dlo_ch[k] >= Bmax)
            B = np.minimum(np.maximum(d, A), B)
    return fwd & bwd


def _schedule(bp64, nv64):
    """Union-over-cores plane covers + engine assignment.

    Returns (istar, plan). plan entries: (k, gi0, gw, gen_eng) where
    [gi0, gi0+gw) is the plane-union i-interval covering all channels and
    cores, and gen_eng is 'act' or 'dve' for the 3 generation ops.
    """
    surv = np.zeros((N, C, NCH), bool)
    for core in range(NCORES):
        surv |= _sample_pruning(bp64, nv64, core * SHARD, (core + 1) * SHARD)
    plane = surv.any(axis=1)  # [N, NCH]
    plane[0] = True           # k=0 writes carry directly; force full cover
    covers = []
    for k in range(N):
        idx = np.nonzero(plane[k])[0]
        if len(idx) == 0:
            covers.append(None)
            continue
        gi0 = idx[0] * CH
        gw = (idx[-1] + 1) * CH - gi0
        covers.append((gi0, gw))

    # stripe boundary: balance DVE vs GpSimd apply-chain time (planes 1..).
    # GpSimd TensorTensor fails the TRN2 ISA engine check (walrus), so the
    # second apply lane is disabled: istar pinned to H (all applies on DVE).
    best = None
    for istar in [H]:
        tD = tG = 0.0
        genw = 0.0
        for k in range(1, N):
            if covers[k] is None:
                continue
            gi0, gw = covers[k]
            wd = max(0, min(gi0 + gw, istar) - gi0)
            wg = max(0, (gi0 + gw) - max(gi0, istar))
            genw += gw
            if wd:
                tD += TTV_BASE + TTV_R * 3 * wd
            if wg:
                tG += TTG_BASE + TTG_R * 3 * wg
        # gens: ACT takes nA of the 3*(N-1) units, DVE the rest; balance
        # act_load = dve_apply + dve_gen_load at the optimum
        units = []
        for k in range(1, N):
            if covers[k] is not None:
                units += [covers[k][1]] * 3
        units.sort()
        # greedy: give DVE the cheapest-for-DVE units until loads equalize
        actL, dveL = 0.0, tD
        for w in sorted(units, reverse=True):
            ca = ACT_BASE + ACT_R * w
            cv = TS_BASE + TS_R * w
            if actL + ca <= dveL + cv:
                actL += ca
            else:
                dveL += cv
        m = max(actL, dveL, tG)
        if best is None or m < best[0]:
            best = (m, istar)
    _, istar = best

    # final gen assignment with the chosen istar
    tD = 0.0
    for k in range(1, N):
        if covers[k] is None:
            continue
        gi0, gw = covers[k]
        wd = max(0, min(gi0 + gw, istar) - gi0)
        if wd:
            tD += TTV_BASE + TTV_R * 3 * wd
    plan = []
    actL, dveL = 0.0, tD
    for k in range(N):
        if covers[k] is None:
            plan.append(None)
            continue
        gi0, gw = covers[k]
        if k == 0:
            plan.append((k, gi0, gw, "act"))
            actL += 3 * (ACT_BASE + ACT_R * H)
            continue
        ca = ACT_BASE + ACT_R * gw
        cv = TS_BASE + TS_R * gw
        if actL + 3 * ca <= dveL + 3 * cv:
            actL += 3 * ca
            plan.append((k, gi0, gw, "act"))
        else:
            dveL += 3 * cv
            plan.append((k, gi0, gw, "dve"))
    return istar, plan


def _build(istar, plan, a_imm):
    """a_imm[k] = fp32 n0_k baked as tensor_scalar immediates."""
    nc = bacc.Bacc("TRN2", target_bir_lowering=False, debug=False)
    acol_d = nc.dram_tensor("acol", [128, N], mybir.dt.float32, kind="ExternalInput")
    ppc_d = nc.dram_tensor("ppc", [128, C * N], mybir.dt.float32, kind="ExternalInput")
    out_d = nc.dram_tensor("out", [128, C * H], mybir.dt.float16, kind="ExternalOutput")
    f32, f16, i32 = mybir.dt.float32, mybir.dt.float16, mybir.dt.int32
    mx, mn = mybir.AluOpType.max, mybir.AluOpType.min
    mult, add = mybir.AluOpType.mult, mybir.AluOpType.add
    ident = mybir.ActivationFunctionType.Identity

    with ExitStack() as ctx:
        tc = ctx.enter_context(tile.TileContext(nc))
        pool = ctx.enter_context(tc.tile_pool(name="main", bufs=1))
        dpool = ctx.enter_context(tc.tile_pool(name="dgen", bufs=8))

        carry = pool.tile([128, C, H], f16)
        acol = pool.tile([128, N], f32)
        ppc = pool.tile([128, C * N], f32)
        ioti = pool.tile([128, H], i32)
        iotf = pool.tile([128, H], f32)
        ioth = pool.tile([128, H], f16)
        nc.gpsimd.dma_start(acol[:], acol_d[:])
        nc.gpsimd.dma_start(ppc[:], ppc_d[:])
        nc.gpsimd.iota(ioti[:], pattern=[[1, H]], base=0, channel_multiplier=0)
        nc.scalar.copy(iotf[:], ioti[:])
        nc.scalar.copy(ioth[:], ioti[:])

        for entry in plan:
            if entry is None:
                continue
            k, gi0, gw, gen_eng = entry
            sc = acol[:, k:k + 1]
            op = mx if k % 2 == 0 else mn
            if k == 0:
                for c in range(C):
                    nc.scalar.activation(carry[:, c, :], iotf[:, :], ident,
                                         bias=ppc[:, c * N: c * N + 1], scale=sc)
                continue
            dbuf = dpool.tile([128, C, H], f16, tag="dbuf")
            for c in range(C):
                bias = ppc[:, c * N + k: c * N + k + 1]
                if gen_eng == "act":
                    nc.scalar.activation(dbuf[:, c, :gw], iotf[:, gi0:gi0 + gw],
                                         ident, bias=bias, scale=sc)
                else:
                    nc.vector.tensor_scalar(dbuf[:, c, :gw], ioth[:, gi0:gi0 + gw],
                                            float(a_imm[k]), bias, mult, add)
            wd = max(0, min(gi0 + gw, istar) - gi0)
            wg = max(0, (gi0 + gw) - max(gi0, istar))
            assert wg == 0, "GpSimd apply lane disabled (TRN2 ISA check)"
            if wd:
                nc.vector.tensor_tensor(carry[:, :, gi0:gi0 + wd],
                                        carry[:, :, gi0:gi0 + wd],
                                        dbuf[:, :, 0:wd], op)

        nc.gpsimd.dma_start(out_d[:], carry[:])
    nc.compile()
    return nc


def _prepare(basepoints, normal_vectors):
    bp = np.asarray(basepoints, np.float32)
    nv = np.asarray(normal_vectors, np.float32)
    bp64 = bp.astype(np.float64)
    nv64 = nv.astype(np.float64)

    istar, plan = _schedule(bp64, nv64)
    cshift = ((np.arange(C)[None, :] - bp64[:, 2:3]) * nv64[:, 2:3]).astype(np.float32)
    nc = _build(istar, plan, nv[:, 0].astype(np.float32))

    acol = np.broadcast_to(nv[:, 0][None, :], (128, N)).copy()  # a_k = n0_k
    js = np.arange(W, dtype=np.float32)
    in_maps = []
    for core in range(NCORES):
        j = js[core * SHARD:(core + 1) * SHARD]                   # [128]
        B = (j[:, None] - bp[None, :, 1]) * nv[None, :, 1]        # [128, N]
        corr = (-bp64[:, 0] * nv64[:, 0])[None, :]
        q = (B.astype(np.float64) + corr).astype(np.float32)      # [128, N]
        ppc = q[:, None, :] + cshift.T[None, :, :]                # [128, C, N]
        in_maps.append({
            "acol": acol,
            "ppc": np.ascontiguousarray(ppc.reshape(128, C * N)),
        })
    return nc, in_maps


def _gather(res):
    out = np.empty((C, H, W), np.float32)
    for core in range(NCORES):
        o = np.asarray(res.results[core]["out"]).reshape(SHARD, C, H)  # [j, c, i]
        out[:, :, core * SHARD:(core + 1) * SHARD] = \
            o.transpose(1, 2, 0).astype(np.float32)
    return out


def kernel(basepoints: np.ndarray, normal_vectors: np.ndarray) -> np.ndarray:
    nc, in_maps = _prepare(basepoints, normal_vectors)
    res = run_bass_kernel_spmd(nc, in_maps, list(range(NCORES)))
    return _gather(res)


def kernel_timed(basepoints: np.ndarray, normal_vectors: np.ndarray):
    """Run with NTFF tracing; returns (exec_time_ns, output, results)."""
    nc, in_maps = _prepare(basepoints, normal_vectors)
    res = run_bass_kernel_spmd(nc, in_maps, list(range(NCORES)), trace=True,
                               trace_cores=list(range(NCORES)))
    return res.exec_time_ns, _gather(res), res
